# revision 28
# baseline (speedup 1.0000x reference)
"""Trainium2 Bass kernel for the sparse-attention block (full attn + window attn + MLP).

Layout strategy (per core, data-parallel over sq_len: 16 frames/core):
  - Residual stream kept FEATURE-major on device: xT [768 x 4096] (f32r).
  - All linears: lhsT = natural [in, out] weight chunks, rhs = feature-major
    activations; fp32r matmuls (full fp32 precision at 4x the fp32 rate).
  - Scores computed in [s x t] orientation (bf16 q/k); softmax denominators via
    ones-selector matmuls; att@v col-tiled so head pairs fill partitions 0..127.
  - Window permutation done by a scattered DMA write after attn1 (64B runs);
    host performs the inverse permutation + transposes (not on the HW path).
"""

import math
import numpy as np

D = 768
NH = 12
HD = 64
T = 256          # tokens per attention group
NF = 16          # frames (or windows) per core
NT = NF * T      # tokens per core
NCORE = 8
DC = 6           # feature chunks (768/128)
FH = 3072        # MLP hidden
FHC = 24         # hidden chunks
EPS = 1e-5

_CACHE = {}


def _build(with_mask: bool, iters: int = 1, qk_bf16: bool = True, contig_m1: bool = False, stages=(1, 2, 3, 4)):
    import contextlib
    from concourse import bacc
    import concourse.mybir as mybir
    import concourse.tile as tile

    f32 = mybir.dt.float32
    f32r = mybir.dt.float32r
    bf16 = mybir.dt.bfloat16
    AF = mybir.ActivationFunctionType
    OP = mybir.AluOpType

    nc = bacc.Bacc("TRN2", target_bir_lowering=False, debug=False, num_devices=NCORE)

    # all private tensors use slab-major layouts so every DMA is 128
    # contiguous per-partition runs (descriptor-count is the DMA bottleneck)
    xt = nc.dram_tensor("xt", [8, 128, DC, 512], f32r, kind="ExternalInput")
    wnames = ["q1", "k1", "v1", "o1", "q2", "k2", "v2", "o2"]
    W = {n: nc.dram_tensor(f"w_{n}", [128, DC * D], f32r, kind="ExternalInput") for n in wnames}
    W["f1"] = nc.dram_tensor("w_f1", [128, DC * FH], bf16, kind="ExternalInput")
    W["f2"] = nc.dram_tensor("w_f2", [128, FHC * D], bf16, kind="ExternalInput")
    # biasd columns: 0:6 qb1, 6:12 kb1, 12:18 qb2, 18:24 kb2, 24:30 c1b,
    #                30:36 c2b, 36:42 f2b, 42:66 f1b
    biasd = nc.dram_tensor("biasd", [128, 66], f32, kind="ExternalInput")
    # lnpd blocks of 768: gz1..bg3; sel at [4608:4736]; onz at [4736:5248]; zz at [5248:5760]
    lnpd = nc.dram_tensor("lnpd", [2, 6 * D + 128 + 1024], f32r, kind="ExternalInput")
    # onesd cols: 0 ones (stat lhsT); 1:3 = [1,0]; 3:5 = [0,1]; 5:517 zeros
    onesd = nc.dram_tensor("onesd", [128, 517], f32r, kind="ExternalInput")
    if with_mask:
        maskd = nc.dram_tensor("maskd", [128, 2, T], f32, kind="ExternalInput")

    ot = nc.dram_tensor("ot", [16, 128, DC, T], f32, kind="ExternalOutput")
    m1t = nc.dram_tensor("m1t", [8, 128, DC, 512], f32r)     # window-pair-major
    m2t = nc.dram_tensor("m2t", [16, 128, DC, T], f32r)      # quarter-slab-major
    # scatter view: [p][wpair][c][u(w%2)][j]  (j = f*16 + hh*4 + ww)
    m1sc = m1t.ap().rearrange("w p c (u j) -> p w c u j", u=2)

    with tile.TileContext(nc) as tc:
        est = contextlib.ExitStack()
        with est:
            est.enter_context(nc.allow_low_precision(
                reason="float32r is 4-byte fp32 on TRN2; full precision measured on HW"))
            cpool = est.enter_context(tc.tile_pool(name="consts", bufs=1))
            spool = est.enter_context(tc.tile_pool(name="small", bufs=2))
            pp = est.enter_context(tc.tile_pool(name="ps", bufs=8, space="PSUM"))

            def psum():
                return pp.tile([128, 512], f32, tag="ps", name="ps")

            # ---------------- constants ----------------
            biasp = cpool.tile([128, 66], f32, tag="biasp")
            nc.sync.dma_start(biasp[:], biasd[:])
            lnp = cpool.tile([2, 6 * D + 128 + 1024], f32r, tag="lnp")
            nc.sync.dma_start(lnp[:], lnpd[:])
            onesp = cpool.tile([128, 517], f32r, tag="onesp")
            nc.sync.dma_start(onesp[:], onesd[:])
            if with_mask:
                mask_sb = cpool.tile([128, 2, T], f32, tag="mask_sb")
                nc.sync.dma_start(mask_sb[:], maskd[:])

            def bias_col(idx, n=1):
                return biasp[:, idx:idx + n]

            def ln_params(i):  # (gz, bg) each [2, 768]
                return (lnp[:, 2 * (i - 1) * D:(2 * i - 1) * D],
                        lnp[:, (2 * i - 1) * D:2 * i * D])

            sel = lnp[:, 6 * D:6 * D + 128]          # [2, 128]
            onz = lnp[:, 6 * D + 128:6 * D + 640]     # [2, 512] rows = [ones ; zeros]
            zz = lnp[:, 6 * D + 640:6 * D + 1152]     # [2, 512] all zeros
            ones_c = onesp[:, 0:1]                    # [128, 1]
            zcols = onesp[:, 5:517]                   # [128, 512] zeros

            # ---------------- helpers ----------------
            def layernorm(xs, a_out, toff, gz, bg):
                """LN over features for T=256 tokens at col offset toff.
                xs, a_out: [128, DC, >=toff+256] f32r tiles."""
                tlen = T
                st_ps = psum()  # row0: [0:256] Sx, [256:512] Sx2
                for c in range(DC):
                    x2 = spool.tile([128, T], f32r, tag="scr", name="x2")
                    nc.vector.tensor_tensor(
                        x2[:, :], xs[:, c, toff:toff + tlen], xs[:, c, toff:toff + tlen], OP.mult)
                    nc.tensor.matmul(st_ps[0:1, 0:tlen], ones_c, xs[:, c, toff:toff + tlen],
                                     start=(c == 0), stop=False)
                    nc.tensor.matmul(st_ps[0:1, tlen:2 * tlen], ones_c, x2[:, :],
                                     start=False, stop=(c == DC - 1))
                # All row-vector scratch lives at partition base 0 (rows 0..1);
                # compute-engine APs must start at a 32-aligned partition.
                t_m = spool.tile([2, 512], f32r, tag="vec", name="t_m", bufs=4)     # row0 = m
                t_e = spool.tile([2, 512], f32r, tag="vec", name="t_e", bufs=4)     # row0 = E[x^2] -> var -> sd
                statr = spool.tile([2, 512], f32r, tag="vec", name="statr", bufs=4)  # [r ; 0]
                mrz = spool.tile([2, 512], f32r, tag="vec", name="mrz", bufs=4)      # [-m*r ; 0]
                nc.vector.tensor_copy(statr[0:2, 0:tlen], zz[:, 0:tlen])
                nc.vector.tensor_copy(mrz[0:2, 0:tlen], zz[:, 0:tlen])
                nc.vector.tensor_scalar_mul(t_m[0:1, 0:tlen], st_ps[0:1, 0:tlen], 1.0 / D)
                nc.vector.tensor_scalar_mul(t_e[0:1, 0:tlen], st_ps[0:1, tlen:2 * tlen], 1.0 / D)
                # stash m^2 in mrz row0 (overwritten later by -m*r)
                nc.vector.tensor_tensor(mrz[0:1, 0:tlen], t_m[0:1, 0:tlen], t_m[0:1, 0:tlen], OP.mult)
                nc.vector.tensor_tensor(t_e[0:1, 0:tlen], t_e[0:1, 0:tlen], mrz[0:1, 0:tlen], OP.subtract)
                nc.vector.tensor_scalar_add(t_e[0:1, 0:tlen], t_e[0:1, 0:tlen], EPS)
                nc.scalar.activation(t_e[0:1, 0:tlen], t_e[0:1, 0:tlen], AF.Sqrt, bias=0.0, scale=1.0)
                nc.vector.reciprocal(statr[0:1, 0:tlen], t_e[0:1, 0:tlen])
                nc.vector.tensor_tensor(mrz[0:1, 0:tlen], t_m[0:1, 0:tlen], statr[0:1, 0:tlen], OP.mult)
                nc.vector.tensor_scalar_mul(mrz[0:1, 0:tlen], mrz[0:1, 0:tlen], -1.0)
                for c in range(DC):
                    gh = psum()  # [:, 0:256] = G = g@r ; [:, 256:512] = H = b@1 + g@(-mr)
                    nc.tensor.matmul(gh[:, 0:tlen], gz[:, 128 * c:128 * (c + 1)],
                                     statr[0:2, 0:tlen], start=True, stop=False)
                    nc.tensor.matmul(gh[:, 256:256 + tlen], bg[:, 128 * c:128 * (c + 1)],
                                     onz[:, 0:tlen], start=False, stop=False)
                    nc.tensor.matmul(gh[:, 256:256 + tlen], gz[:, 128 * c:128 * (c + 1)],
                                     mrz[0:2, 0:tlen], start=False, stop=True)
                    tmp = spool.tile([128, T], f32, tag="scr", name="lntmp")
                    nc.vector.tensor_tensor(tmp[:, :], xs[:, c, toff:toff + tlen],
                                            gh[:, 0:tlen], OP.mult)
                    nc.vector.tensor_tensor(a_out[:, c, toff:toff + tlen], tmp[:, :],
                                            gh[:, 256:256 + tlen], OP.add)

            def attention_stage(stage):
                """stage 1: xt -> m1t (scatter); stage 2: m1t -> m2t."""
                st = contextlib.ExitStack()
                with st:
                    wpool = st.enter_context(tc.tile_pool(name=f"wts{stage}", bufs=1))
                    apool = st.enter_context(tc.tile_pool(name=f"acts{stage}", bufs=1))
                    dpool = st.enter_context(tc.tile_pool(name=f"dbl{stage}", bufs=2))
                    hpool = st.enter_context(tc.tile_pool(name=f"heads{stage}", bufs=2))

                    def load_w(nm, tag):
                        w_ = wpool.tile([128, DC, D], f32r, tag=tag, name=tag)
                        nc.sync.dma_start(w_[:], W[nm].ap().rearrange("p (c o) -> p c o", c=DC))
                        return w_

                    # zero-padded v buffer: per s-chunk and head pair,
                    # cols [0:128] = [v_even | 0], cols [128:256] = [0 | v_odd]
                    vzp = apool.tile([128, 4, 6, 256], f32r, tag="vzp", name="vzp")
                    for z in range(12):
                        nc.vector.tensor_copy(
                            vzp[:].rearrange("p a b c -> p (a b c)")[:, 512 * z:512 * (z + 1)],
                            zcols[:, :])
                    sfx = "1" if stage == 1 else "2"
                    wq, wk = load_w("q" + sfx, "wq"), load_w("k" + sfx, "wk")
                    wv, wo = load_w("v" + sfx, "wv"), load_w("o" + sfx, "wo")
                    qb = bias_col(0 if stage == 1 else 12, DC)
                    kb = bias_col(6 if stage == 1 else 18, DC)
                    cb = bias_col(24 if stage == 1 else 30, DC)
                    gz, bg = ln_params(stage)
                    src_t = xt if stage == 1 else m1t

                    for fp in range(NF // 2):  # frame pairs
                        fpi = fp % 2
                        if stage == 1 and fpi == 0:
                            o4 = apool.tile([128, DC, 1024], f32r, tag="o4", name="o4")
                        xs = dpool.tile([128, DC, 512], f32r, tag="xs", name="xs")
                        nc.sync.dma_start(xs[:], src_t.ap()[fp])
                        a1 = apool.tile([128, DC, 512], f32r, tag="big", name="a1")
                        for half in range(2):
                            layernorm(xs, a1, 256 * half, gz, bg)

                        # qT, kT feature-major [128(hd), DC, 512] in bf16
                        qkdt = bf16 if qk_bf16 else f32r
                        qT = apool.tile([128, DC, 512], qkdt, tag="qT", name="qT")
                        kT = apool.tile([128, DC, 512], qkdt, tag="kT", name="kT")
                        for dst, w_, b_ in ((qT, wq, qb), (kT, wk, kb)):
                            for oc in range(DC):
                                ps = psum()
                                for c in range(DC):
                                    nc.tensor.matmul(ps[:, :], w_[:, c, 128 * oc:128 * (oc + 1)],
                                                     a1[:, c, :], start=(c == 0), stop=(c == DC - 1))
                                nc.scalar.activation(dst[:, oc, :], ps[:, :], AF.Identity,
                                                     bias=b_[:, oc:oc + 1], scale=1.0)
                        # v into the zero-padded pair blocks of vzp
                        for sc in range(4):
                            for half in range(2):
                                ps = psum()
                                for c in range(DC):
                                    nc.tensor.matmul(ps[:, 0:384],
                                                     a1[:, c, 128 * sc:128 * (sc + 1)],
                                                     wv[:, c, 384 * half:384 * (half + 1)],
                                                     start=(c == 0), stop=(c == DC - 1))
                                psv = ps[:, 0:384].rearrange("p (hp par d) -> p hp par d", par=2, d=64)
                                # even heads -> block0 cols 0:64 ; odd heads -> block1 cols 64:128
                                nc.vector.tensor_copy(
                                    vzp[:, sc, 3 * half:3 * (half + 1), 0:64], psv[:, :, 0, :])
                                nc.vector.tensor_copy(
                                    vzp[:, sc, 3 * half:3 * (half + 1), 192:256], psv[:, :, 1, :])

                        yT = apool.tile([128, DC, 512], f32r, tag="big", name="yT")
                        for f in range(2):
                            toff = 256 * f
                            for hp in range(6):
                                sps = [psum(), psum()]  # per head [128 x 512], halves = s-chunks
                                for sc2 in range(2):
                                    for hh in range(2):
                                        nc.tensor.matmul(
                                            sps[hh][:, 256 * sc2:256 * (sc2 + 1)],
                                            kT[64 * hh:64 * (hh + 1), hp, toff + 128 * sc2:toff + 128 * (sc2 + 1)],
                                            qT[64 * hh:64 * (hh + 1), hp, toff:toff + 256],
                                            start=(sc2 == 0), stop=(sc2 == 1),
                                            tile_position=(64 * hh, 0))
                                att = [None, None]
                                for hh in range(2):
                                    if with_mask and stage == 1:
                                        nc.vector.tensor_tensor(
                                            sps[hh][:, :], sps[hh][:, :],
                                            mask_sb[:].rearrange("p s t -> p (s t)"), OP.add)
                                    att[hh] = hpool.tile([128, 2, T], f32r, tag="att", name="att")
                                    nc.scalar.activation(
                                        att[hh][:].rearrange("p s t -> p (s t)"), sps[hh][:, :],
                                        AF.Exp, bias=0.0, scale=1.0 / math.sqrt(HD))
                                # denominators share the r2 bank: pd rows at cols [256:512]
                                r2ps = psum()
                                for hh in range(2):
                                    for sc2 in range(2):
                                        nc.tensor.matmul(
                                            r2ps[0:2, 256:256 + T], onesp[:, 1 + 2 * hh:3 + 2 * hh],
                                            att[hh][:, sc2, :],
                                            start=(hh == 0 and sc2 == 0), stop=(hh == 1 and sc2 == 1))
                                # att @ v with zero-padded blocks: rows 0:64 h0, 64:128 h1
                                yps = psum()
                                for sc2 in range(2):
                                    for hh in range(2):
                                        nc.tensor.matmul(
                                            yps[:, 0:T],
                                            vzp[:, 2 * f + sc2, hp, 128 * hh:128 * (hh + 1)],
                                            att[hh][:, sc2, :],
                                            start=(sc2 == 0 and hh == 0), stop=(sc2 == 1 and hh == 1))
                                pdr = spool.tile([3, 512], f32r, tag="vec", name="pdr", bufs=4)
                                nc.vector.reciprocal(pdr[0:2, 0:T], r2ps[0:2, 256:256 + T])
                                nc.tensor.matmul(r2ps[:, 0:T], sel, pdr[0:2, 0:T], start=True, stop=True)
                                r2sb = spool.tile([128, T], f32, tag="scr", name="r2sb")
                                nc.scalar.copy(r2sb[:], r2ps[:, 0:T])
                                nc.vector.tensor_tensor(yT[:, hp, toff:toff + 256],
                                                        yps[:, 0:T], r2sb[:], OP.mult)

                        # O-projection + residual + bias -> feature-major out.
                        # Stage 1 writes into o4 (4 frames) with columns in
                        # (w, f2, hh, ww) order so the window scatter has 256B runs.
                        if stage == 1:
                            for ec in range(DC):
                                ps = psum()
                                for hc in range(DC):
                                    nc.tensor.matmul(ps[:, :], wo[:, hc, 128 * ec:128 * (ec + 1)],
                                                     yT[:, hc, :], start=(hc == 0), stop=(hc == DC - 1))
                                o4v = o4[:, ec, :].rearrange(
                                    "p (hi wi f2 hh ww) -> p hi f2 hh wi ww", hi=4, wi=4, f2=4, hh=4)
                                for f in range(2):
                                    f2 = 2 * fpi + f
                                    for hi in range(4):
                                        sl_ = slice(256 * f + 64 * hi, 256 * f + 64 * (hi + 1))
                                        nc.vector.tensor_tensor(
                                            o4v[:, hi, f2],
                                            ps[:, sl_].rearrange(
                                                "p (hh wi ww) -> p hh wi ww", hh=4, wi=4),
                                            xs[:, ec, sl_].rearrange(
                                                "p (hh wi ww) -> p hh wi ww", hh=4, wi=4),
                                            OP.add)
                            if fpi == 1:
                                g4 = fp // 2
                                for ec in range(DC):
                                    nc.vector.tensor_scalar_add(
                                        o4[:, ec, :], o4[:, ec, :], cb[:, ec:ec + 1])
                                    o4s = o4[:, ec, :].rearrange(
                                        "p (w8 u run) -> p w8 u run", w8=8, u=2)
                                    for w2 in range(2):
                                        nc.sync.dma_start(
                                            m1sc[:, :, ec, w2, 64 * g4:64 * (g4 + 1)],
                                            o4s[:, :, w2, :])
                        else:
                            o1 = apool.tile([128, DC, 512], f32r, tag="big", name="o1")
                            for ec in range(DC):
                                ps = psum()
                                for hc in range(DC):
                                    nc.tensor.matmul(ps[:, :], wo[:, hc, 128 * ec:128 * (ec + 1)],
                                                     yT[:, hc, :], start=(hc == 0), stop=(hc == DC - 1))
                                nc.vector.tensor_tensor(o1[:, ec, :], ps[:, :], xs[:, ec, :], OP.add)
                                nc.vector.tensor_scalar_add(o1[:, ec, :], o1[:, ec, :], cb[:, ec:ec + 1])
                            nc.sync.dma_start(m2t.ap()[2 * fp], o1[:, :, 0:256])
                            nc.sync.dma_start(m2t.ap()[2 * fp + 1], o1[:, :, 256:512])

            def mlp_stage():
                st = contextlib.ExitStack()
                with st:
                    wpool = st.enter_context(tc.tile_pool(name="wtsm", bufs=1))
                    dpool = st.enter_context(tc.tile_pool(name="dblm", bufs=2))
                    gpool = st.enter_context(tc.tile_pool(name="gm", bufs=4))
                    w1 = wpool.tile([128, DC, FH], bf16, tag="w1", name="w1")
                    nc.sync.dma_start(w1[:], W["f1"].ap().rearrange("p (c o) -> p c o", c=DC))
                    w2 = wpool.tile([128, FHC, D], bf16, tag="w2", name="w2")
                    nc.sync.dma_start(w2[:], W["f2"].ap().rearrange("p (c o) -> p c o", c=FHC))
                    gz, bg = ln_params(3)
                    f1b = bias_col(42, FHC)
                    f2b = bias_col(36, DC)
                    for sl in range(NF):
                        xs3 = dpool.tile([128, DC, T], f32r, tag="xs3", name="xs3")
                        nc.sync.dma_start(xs3[:], m2t.ap()[sl])
                        h3 = dpool.tile([128, DC, T], bf16, tag="h3", name="h3")
                        layernorm(xs3, h3, 0, gz, bg)
                        # fc1 -> gelu -> fc2 accumulate (z held in 3 PSUM banks)
                        zps = [psum(), psum(), psum()]
                        for oc in range(FHC):
                            ps1 = psum()
                            for c in range(DC):
                                nc.tensor.matmul(ps1[:, 0:T], w1[:, c, 128 * oc:128 * (oc + 1)],
                                                 h3[:, c, :], start=(c == 0), stop=(c == DC - 1))
                            g1c = gpool.tile([128, T], bf16, tag="g1c", name="g1c")
                            nc.scalar.activation(g1c[:], ps1[:, 0:T], AF.Gelu,
                                                 bias=f1b[:, oc:oc + 1], scale=1.0)
                            for ec in range(DC):
                                nc.tensor.matmul(
                                    zps[ec // 2][:, 256 * (ec % 2):256 * (ec % 2) + T],
                                    w2[:, oc, 128 * ec:128 * (ec + 1)], g1c[:],
                                    start=(oc == 0 and ec % 2 == 0),
                                    stop=(oc == FHC - 1 and ec % 2 == 1))
                        oo = dpool.tile([128, DC, T], f32, tag="oo", name="oo")
                        for ec in range(DC):
                            nc.vector.tensor_tensor(
                                oo[:, ec, :], zps[ec // 2][:, 256 * (ec % 2):256 * (ec % 2) + T],
                                xs3[:, ec, :], OP.add)
                            nc.vector.tensor_scalar_add(oo[:, ec, :], oo[:, ec, :], f2b[:, ec:ec + 1])
                        nc.sync.dma_start(ot.ap()[sl], oo[:])

            def body(iv=None):
                if 1 in stages:
                    attention_stage(1)
                if 2 in stages:
                    attention_stage(2)
                if 3 in stages:
                    mlp_stage()

            if iters == 1:
                body()
            else:
                with tc.For_i(0, iters, 1) as iv:
                    body(iv)

    nc.compile()
    return nc


def _host_prep(inputs):
    """Build per-core input maps."""
    x = np.asarray(inputs["x"], np.float32)
    mask = np.asarray(inputs["mask"])
    with_mask = not bool((mask == 1).all())
    f32 = np.float32

    com = {}
    for n, key in [("q1", "q1_w"), ("k1", "k1_w"), ("v1", "v1_w"), ("o1", "o1_w"),
                   ("q2", "q2_w"), ("k2", "k2_w"), ("v2", "v2_w"), ("o2", "o2_w"),
                   ("f1", "fc1_w"), ("f2", "fc2_w")]:
        w = np.asarray(inputs[key], f32)
        kc = w.shape[0] // 128
        wp = np.ascontiguousarray(
            w.reshape(kc, 128, w.shape[1]).transpose(1, 0, 2).reshape(128, -1))
        if n in ("f1", "f2"):
            import ml_dtypes
            wp = wp.astype(ml_dtypes.bfloat16)
        com[f"w_{n}"] = wp

    def chunks(vec, w):
        return np.asarray(vec, f32).reshape(w, 128).T

    biasp = np.zeros((128, 66), f32)
    biasp[:, 0:6] = chunks(inputs["q1_b"], DC)
    biasp[:, 6:12] = chunks(inputs["k1_b"], DC)
    biasp[:, 12:18] = chunks(inputs["q2_b"], DC)
    biasp[:, 18:24] = chunks(inputs["k2_b"], DC)
    c1 = np.asarray(inputs["v1_b"], f32) @ np.asarray(inputs["o1_w"], f32) + np.asarray(inputs["o1_b"], f32)
    c2 = np.asarray(inputs["v2_b"], f32) @ np.asarray(inputs["o2_w"], f32) + np.asarray(inputs["o2_b"], f32)
    biasp[:, 24:30] = chunks(c1, DC)
    biasp[:, 30:36] = chunks(c2, DC)
    biasp[:, 36:42] = chunks(inputs["fc2_b"], DC)
    biasp[:, 42:66] = chunks(inputs["fc1_b"], FHC)
    com["biasd"] = biasp

    lnp = np.zeros((2, 6 * D + 128 + 1024), f32)
    for i, (g, b) in [(1, ("ln1_g", "ln1_b")), (2, ("ln2_g", "ln2_b")), (3, ("ln3_g", "ln3_b"))]:
        gv = np.asarray(inputs[g], f32)
        bv = np.asarray(inputs[b], f32)
        lnp[0, 2 * (i - 1) * D:(2 * i - 1) * D] = gv      # gz row0 = g
        lnp[0, (2 * i - 1) * D:2 * i * D] = bv            # bg row0 = b
        lnp[1, (2 * i - 1) * D:2 * i * D] = gv            # bg row1 = g
    lnp[0, 6 * D:6 * D + 64] = 1.0                         # sel row0
    lnp[1, 6 * D + 64:6 * D + 128] = 1.0                   # sel row1
    lnp[0, 6 * D + 128:6 * D + 640] = 1.0                  # onz row0 = ones
    com["lnpd"] = lnp

    onesd = np.zeros((128, 517), f32)
    onesd[:, 0] = 1.0
    onesd[:, 1] = 1.0   # [1,0] pattern cols 1:3
    onesd[:, 4] = 1.0   # [0,1] pattern cols 3:5
    com["onesd"] = onesd

    if with_mask:
        madd = (mask.reshape(T, T).astype(f32) - 1.0) * 1e9   # [t, s] additive
        maskT = madd.T                                        # [s, t]
        com["maskd"] = np.ascontiguousarray(maskT.reshape(2, 128, T).transpose(1, 0, 2))

    in_maps = []
    for c in range(NCORE):
        shard = x[NF * c:NF * (c + 1)]                  # [16, 256, 768]
        xT = shard.reshape(NT, D).T                     # [768, 4096]
        m = dict(com)
        m["xt"] = np.ascontiguousarray(
            xT.reshape(DC, 128, 8, 512).transpose(2, 1, 0, 3))   # [8,128,6,512]
        in_maps.append(m)
    return in_maps, with_mask


def _host_post(results, dtype):
    """results: list of 8 dicts with 'ot' [768, 4096] window-major."""
    out = np.empty((NCORE * NF, T, D), dtype)
    for c, r in enumerate(results):
        o = r["ot"]                               # [16, 128, 6, 256] slab-major
        o = o.transpose(2, 1, 0, 3).reshape(D, NT)  # feature-major window order
        o = o.reshape(D, 4, 4, NF, 4, 4)          # (d, hi, wi, f, hh, ww)
        o = o.transpose(3, 1, 4, 2, 5, 0)         # (f, hi, hh, wi, ww, d)
        out[NF * c:NF * (c + 1)] = o.reshape(NF, T, D)
    return out


def kernel(**inputs) -> np.ndarray:
    from concourse.bass_utils import run_bass_kernel_spmd

    in_maps, with_mask = _host_prep(inputs)
    key = ("k", with_mask)
    if key not in _CACHE:
        _CACHE[key] = _build(with_mask)
    nc = _CACHE[key]
    res = run_bass_kernel_spmd(nc, in_maps, core_ids=list(range(NCORE)))
    return _host_post(res.results, np.asarray(inputs["x"]).dtype)
